# revision 20
# baseline (speedup 1.0000x reference)
"""Causal multi-head self-attention on 8 Trainium2 NeuronCores (Bass/Tile).

Problem (hardcoded shapes): x [2, 2048, 768] f32, 12 heads of dim 64.
    qkv = x @ Wqkv + bqkv ; per-head causal softmax(q k^T / 8) @ v ; out @ Wproj + bproj

Sharding: 8 cores = 2 batches x 4 head-groups (3 heads each). Each core computes
its heads' QKV, attention, and a partial output projection (its rows of Wproj).
Host sums the 4 partial projections per batch and adds bproj.

v2 design (all matmuls bf16, softmax-pipelined):
  - Inputs arrive bf16 (host-cast); x pre-transposed to xT [768, 2048].
  - qkT chunks {q0q1, k0k1, q2k2} each [128, 2048] via 6-step PSUM chains;
    PSUM->SBUF move on DVE carries the bias add (bf16 out).
  - v natural [2048, 3*65] (ones column for row sums), bias in the DVE move.
  - Attention, query-block 512, keys tiled by 128 (round r = (qc, kk)):
      scores^T on PE (h0/h1 at partition offsets 0/64 -> row-group paired,
      near-concurrent), ONE exp per score tile on ACT (bias = key mask),
      causal tri mask on GpSimd (diag rounds), PV accumulation into [65,512]
      PSUM per head.  Software pipelined so the PE never waits at queue head:
      per round emit scores(r), pv01(r-1), s1(r+1), pv2(r).
  - Background GEMM chains (remaining qk/v slices, projection tiles) are
    dispensed into the attention rounds' PE slack.
  - Projection y = A @ Wp accumulated in PSUM, moved to SBUF as bf16, DMA'd
    out per seq tile; host upcasts, sums the 4 partial projections, adds bproj.
PSUM: sA [128,1024] + sB [128,512] single-buffered, 3x oacc [65,512], 2 bg.
"""
import os
import numpy as np
import ml_dtypes

import concourse.bass as bass
import concourse.mybir as mybir
import concourse.tile as tile
from concourse import bacc
from concourse.bass_utils import run_bass_kernel_spmd
from concourse.masks import make_upper_triangular

f32 = mybir.dt.float32
f32r = mybir.dt.float32r
bf16 = mybir.dt.bfloat16

T = 2048          # sequence length
H = 768           # model dim
HD = 64           # head dim
NT = T // 128     # 16 key tiles
KH = H // 128     # 6 contraction chunks

_cache = {}
last_results = None


def _build():
    nc = bacc.Bacc("TRN2", target_bir_lowering=False, debug=False)

    xs_d = [nc.dram_tensor(f"xs{n}", [H, 512], bf16, kind="ExternalInput")
            for n in range(4)]
    wqk_d = nc.dram_tensor("wqk", [H, 384], bf16, kind="ExternalInput")
    wv_d = nc.dram_tensor("wv", [H, 192], bf16, kind="ExternalInput")
    wp_d = nc.dram_tensor("wp", [192, H], bf16, kind="ExternalInput")
    smalls_d = nc.dram_tensor("smalls", [128, 3 + NT + 192], f32, kind="ExternalInput")
    y_d = nc.dram_tensor("y", [T, H], bf16, kind="ExternalOutput")

    Exp = mybir.ActivationFunctionType.Exp
    mult = mybir.AluOpType.mult
    add = mybir.AluOpType.add

    with tile.TileContext(nc) as tc:
        with (
            tc.tile_pool(name="singles", bufs=1) as singles,
            tc.tile_pool(name="big", bufs=1) as big,
            tc.tile_pool(name="pt", bufs=2) as ptp,
            tc.tile_pool(name="small", bufs=2) as small,
            tc.tile_pool(name="yout", bufs=2) as ytp,
            tc.tile_pool(name="sA", bufs=1, space="PSUM") as sAp,
            tc.tile_pool(name="sB", bufs=1, space="PSUM") as sBp,
            tc.tile_pool(name="acc", bufs=3, space="PSUM") as accp,
            tc.tile_pool(name="bg", bufs=2, space="PSUM") as bgp,
        ):
            # ---- input DMAs: smalls+weights on the gpsimd queue, x on sync ----
            smalls = singles.tile([128, 3 + NT + 192], f32)
            nc.gpsimd.dma_start(out=smalls[:], in_=smalls_d.ap())
            bqk = smalls[:, 0:3]
            maskcolD = smalls[:, 3:3 + NT]
            bvb = smalls[:, 3 + NT:3 + NT + 192]
            maskcol = singles.tile([128, NT], f32)
            nc.vector.tensor_copy(maskcol[:], maskcolD)
            wqk = singles.tile([128, KH, 384], bf16)
            nc.gpsimd.dma_start(out=wqk[:], in_=wqk_d.ap().rearrange("(a p) m -> p a m", p=128))
            xT = big.tile([128, KH, T], bf16)
            for n in range(4):
                eng = nc.sync if n % 2 == 0 else nc.scalar
                eng.dma_start(out=xT[:, :, n * 512:(n + 1) * 512],
                              in_=xs_d[n].ap().rearrange("(a p) m -> p a m", p=128))
            wv = singles.tile([128, KH, 192], bf16)
            nc.gpsimd.dma_start(out=wv[:], in_=wv_d.ap().rearrange("(a p) m -> p a m", p=128))
            wp0 = singles.tile([128, H], bf16)
            nc.gpsimd.dma_start(out=wp0[:], in_=wp_d.ap()[0:128, :])
            wp1 = singles.tile([64, H], bf16)
            nc.gpsimd.dma_start(out=wp1[:], in_=wp_d.ap()[128:192, :])

            # ---- constants ----
            trif = singles.tile([128, 128], f32)
            make_upper_triangular(nc, trif[:], val=1.0, diag=True)
            tri01 = singles.tile([128, 128], bf16)
            nc.vector.tensor_copy(tri01[:], trif[:])
            ones128 = singles.tile([128, 1], bf16)
            nc.vector.memset(ones128[:], 1.0)
            onesr1 = singles.tile([1, 64], f32)
            nc.vector.memset(onesr1[:], 1.0)

            # ---- persistent activations ----
            qkA = big.tile([128, T], bf16)    # q0 (rows 0:64), q1 (64:128)
            qkB = big.tile([128, T], bf16)    # k0, k1
            qkC2 = big.tile([128, T], bf16)   # q2 (0:64), k2 (64:128)
            k2c = big.tile([64, T], bf16)     # k2 moved to partitions 0:64
            vsb = big.tile([128, NT, 3, HD + 1], bf16)
            nc.vector.tensor_copy(vsb[:, :, :, HD:HD + 1],
                                  ones128[:, 0:1].to_broadcast((128, NT, 3, 1)))
            atA = big.tile([128, T], bf16)    # A^T h0 (0:64), h1 (64:128)
            atB = big.tile([64, T], bf16)     # h2

            # ---- chain emitters ----
            def qk_chain(ci, n, dst):
                ns = slice(n * 512, (n + 1) * 512)
                ps = bgp.tile([128, 512], f32, tag="bg")
                for k in range(KH):
                    nc.tensor.matmul(ps[:], lhsT=wqk[:, k, ci * 128:(ci + 1) * 128],
                                     rhs=xT[:, k, ns], start=(k == 0), stop=(k == KH - 1))
                nc.vector.tensor_tensor(dst[:, ns], ps[:],
                                        bqk[:, ci:ci + 1].to_broadcast((128, 512)), add)

            def k2copy(n):
                ns = slice(n * 512, (n + 1) * 512)
                nc.vector.tensor_copy(k2c[:, ns], qkC2[64:128, ns])

            def v_chain(t):
                ps = bgp.tile([128, 512], f32, tag="bg")
                for k in range(KH):
                    nc.tensor.matmul(ps[:, 0:192], lhsT=xT[:, k, t * 128:(t + 1) * 128],
                                     rhs=wv[:, k, :], start=(k == 0), stop=(k == KH - 1))
                nc.vector.tensor_tensor(
                    vsb[:, t, :, 0:HD],
                    ps[:, 0:192].rearrange("p (h d) -> p h d", h=3),
                    bvb.rearrange("p (h d) -> p h d", h=3), add)

            Copy = mybir.ActivationFunctionType.Copy

            def proj_tile(t, act_half=False):
                ts = slice(t * 128, (t + 1) * 128)
                yt = ytp.tile([128, H], bf16, tag="yt")
                for hi, hs in enumerate((slice(0, 384), slice(384, 768))):
                    ps = bgp.tile([128, 512], f32, tag="bg")
                    nc.tensor.matmul(ps[:, 0:384], lhsT=atA[:, ts], rhs=wp0[:, hs],
                                     start=True, stop=False)
                    nc.tensor.matmul(ps[:, 0:384], lhsT=atB[:, ts], rhs=wp1[:, hs],
                                     start=False, stop=True)
                    if act_half and hi == 0:
                        nc.scalar.activation(out=yt[:, hs], in_=ps[:, 0:384], func=Copy)
                    else:
                        nc.vector.tensor_copy(yt[:, hs], ps[:, 0:384])
                nc.sync.dma_start(out=y_d.ap()[ts, :], in_=yt[:])

            # ---- background work queue: (cost_ns, emit_fn) ----
            # KERNEL_BG_MASK: bit0 = qk chains + k2copies in bg, bit1 = v chains in bg
            bgm = int(os.environ.get("KERNEL_BG_MASK", "3"))
            bgq = []
            front = []
            (bgq if bgm & 2 else front).append((700, lambda: None))
            bgq.pop() if bgm & 2 else front.pop()
            for t in range(4):
                (bgq if bgm & 2 else front).append((700, lambda t=t: v_chain(t)))
            for n in range(1, 4):
                (bgq if bgm & 1 else front).append((1500, lambda n=n: qk_chain(0, n, qkA)))
                (bgq if bgm & 1 else front).append((1500, lambda n=n: qk_chain(1, n, qkB)))
                (bgq if bgm & 1 else front).append((1500, lambda n=n: qk_chain(2, n, qkC2)))
                (bgq if bgm & 1 else front).append((100, lambda n=n: k2copy(n)))
                for t in range(4 * n, 4 * n + 4):
                    (bgq if bgm & 2 else front).append((700, lambda t=t: v_chain(t)))

            # ---- HAM warm-up: dummy matmuls while input DMAs stream ----
            wsrc = singles.tile([128, 512], bf16)
            nc.vector.memset(wsrc[:], 0.25)
            for _w in range(10):
                wps = bgp.tile([128, 512], f32, tag="bg", name="warm")
                nc.tensor.matmul(wps[:], lhsT=wsrc[:, 0:128], rhs=wsrc[:],
                                 start=True, stop=True)

            # ---- front: A0/B0 interleaved per contraction chunk ----
            psA0 = bgp.tile([128, 512], f32, tag="bg", name="psA0")
            psB0 = bgp.tile([128, 512], f32, tag="bg", name="psB0")
            for k in range(KH):
                nc.tensor.matmul(psA0[:], lhsT=wqk[:, k, 0:128], rhs=xT[:, k, 0:512],
                                 start=(k == 0), stop=(k == KH - 1))
                nc.tensor.matmul(psB0[:], lhsT=wqk[:, k, 128:256], rhs=xT[:, k, 0:512],
                                 start=(k == 0), stop=(k == KH - 1))
            nc.vector.tensor_tensor(qkA[:, 0:512], psA0[:],
                                    bqk[:, 0:1].to_broadcast((128, 512)), add)
            nc.vector.tensor_tensor(qkB[:, 0:512], psB0[:],
                                    bqk[:, 1:2].to_broadcast((128, 512)), add)
            for _c, _fn in front:
                _fn()

            # ---- attention rounds ----
            rounds = [(qc, kk) for qc in range(4) for kk in range(4 * qc + 4)]
            NR = len(rounds)
            sAt = [None] * NR
            sBt = [None] * NR
            pt2t = [None] * NR
            pt1t = [None] * NR
            oaccs = [None, None, None]

            def off_of(i):
                qc, kk = rounds[i]
                return max(0, kk * 128 - qc * 512)

            def emit_s1(i):
                qc, kk = rounds[i]
                base, qlo, off = qc * 512, kk * 128, off_of(i)
                sBt[i] = sBp.tile([128, 512], f32, tag="sB", name="sB")
                nc.tensor.matmul(sBt[i][:, off:512], lhsT=k2c[:, qlo:qlo + 128],
                                 rhs=qkC2[0:64, base + off:base + 512],
                                 start=True, stop=True)

            def emit_s2(i):
                qc, kk = rounds[i]
                base, qlo, off = qc * 512, kk * 128, off_of(i)
                sAt[i] = sAp.tile([128, 1024], f32, tag="sA", name="sA")
                nc.tensor.matmul(sAt[i][:, off:512], lhsT=qkB[0:64, qlo:qlo + 128],
                                 rhs=qkA[0:64, base + off:base + 512],
                                 start=True, stop=True)
                nc.tensor.matmul(sAt[i][:, 512 + off:1024], lhsT=qkB[64:128, qlo:qlo + 128],
                                 rhs=qkA[64:128, base + off:base + 512],
                                 start=True, stop=True)

            def emit_exps(i):
                qc, kk = rounds[i]
                off = off_of(i)
                pt2t[i] = ptp.tile([128, 1024], bf16, tag="pt2", name="pt2")
                nc.scalar.activation(out=pt2t[i][:, off:1024], in_=sAt[i][:, off:1024],
                                     func=Exp, bias=maskcol[:, kk:kk + 1], scale=1.0)
                pt1t[i] = ptp.tile([128, 512], bf16, tag="pt1", name="pt1")
                nc.scalar.activation(out=pt1t[i][:, off:512], in_=sBt[i][:, off:512],
                                     func=Exp, bias=maskcol[:, kk:kk + 1], scale=1.0)

            def is_diag(i):
                qc, kk = rounds[i]
                return kk * 128 >= qc * 512

            def emit_tri1(i):
                d = off_of(i)
                nc.vector.tensor_tensor(pt1t[i][:, d:d + 128], pt1t[i][:, d:d + 128],
                                        tri01[:], mult)

            def emit_tri2(i):
                d = off_of(i)
                for o2 in (0, 512):
                    nc.vector.tensor_tensor(pt2t[i][:, o2 + d:o2 + d + 128],
                                            pt2t[i][:, o2 + d:o2 + d + 128],
                                            tri01[:], mult)

            def emit_pv(i, h):
                qc, kk = rounds[i]
                off = off_of(i)
                pt = pt1t[i] if h == 2 else pt2t[i]
                o2 = 512 if h == 1 else 0
                nc.tensor.matmul(oaccs[h][0:HD + 1, off:512],
                                 lhsT=vsb[:, kk, h, :],
                                 rhs=pt[:, o2 + off:o2 + 512] if h != 2 else pt[:, off:512],
                                 start=(kk == 0), stop=(kk == 4 * qc + 3),
                                 skip_group_check=True)

            def new_oaccs():
                for h in range(3):
                    oaccs[h] = accp.tile([HD + 1, 512], f32, tag="acc", name=f"oacc{h}")

            at_of = [(atA, 0), (atA, 64), (atB, 0)]

            def norm_pre(qc):
                rbss = []
                for h in (0, 1, 2):
                    srow = small.tile([1, 512], f32, tag="srow", name=f"srow{h}")
                    nc.vector.tensor_copy(srow[:], oaccs[h][HD:HD + 1, :])
                    rrow = small.tile([1, 512], f32, tag="rrow", name=f"rrow{h}")
                    nc.vector.reciprocal_approx_fast(rrow[:], srow[:])
                    rbs = small.tile([64, 512], f32, tag=f"rbs{h}", name=f"rbs{h}")
                    nc.gpsimd.partition_broadcast(rbs[:], rrow[:])
                    rbss.append((h, rbs))
                return rbss

            def norm_mul(qc, rbss):
                base = qc * 512
                for h, rbs in rbss:
                    at_t, at_o = at_of[h]
                    nc.vector.tensor_tensor(at_t[at_o:at_o + HD, base:base + 512],
                                            oaccs[h][0:HD, :], rbs[:], mult)

            def normalize(qc):
                norm_mul(qc, norm_pre(qc))

            debt = [0.0]

            def drain_bg(budget):
                debt[0] += budget
                while bgq and debt[0] >= bgq[0][0]:
                    cost, fn = bgq.pop(0)
                    debt[0] -= cost
                    fn()

            if int(os.environ.get("KERNEL_PIPELINE", "1")):
                if int(os.environ.get("KERNEL_NOBG", "0")):
                    drain_bg(1e9)
                new_oaccs()
                for i in range(NR):
                    qc, kk = rounds[i]
                    first_of_qc = (kk == 0)
                    emit_s2(i)
                    if i == 0:
                        qk_chain(2, 0, qkC2)
                        k2copy(0)
                        emit_s1(0)
                    if first_of_qc and qc > 0:
                        emit_pv(i - 1, 0)
                        emit_pv(i - 1, 1)
                        emit_pv(i - 1, 2)
                        normalize(qc - 1)
                        for t in range(4 * (qc - 1), 4 * qc):
                            bgq.append((900, lambda t=t: proj_tile(t)))
                        new_oaccs()
                    emit_exps(i)
                    if is_diag(i):
                        emit_tri1(i)
                    drain_bg(2600 if i < 4 else (800 if i >= 24 else 600))
                    if i > 0 and not (first_of_qc and qc > 0):
                        emit_pv(i - 1, 0)
                        emit_pv(i - 1, 1)
                        emit_pv(i - 1, 2)
                    if i + 1 < NR:
                        emit_s1(i + 1)
                    if is_diag(i):
                        emit_tri2(i)
                emit_pv(NR - 1, 0)
                emit_pv(NR - 1, 1)
                emit_pv(NR - 1, 2)
                rbss3 = norm_pre(3)
                drain_bg(1e9)
                for _w in range(6):
                    wps = bgp.tile([128, 512], f32, tag="bg", name="warm2")
                    nc.tensor.matmul(wps[:], lhsT=wsrc[:, 0:128], rhs=wsrc[:],
                                     start=True, stop=True)
                norm_mul(3, rbss3)
                for t in range(12, 16):
                    proj_tile(t, act_half=True)
            else:
                # sequential debug mode: all chains first, plain rounds
                drain_bg(1e9)
                for i in range(NR):
                    qc, kk = rounds[i]
                    if kk == 0:
                        new_oaccs()
                    emit_s2(i)
                    emit_s1(i)
                    emit_exps(i)
                    if is_diag(i):
                        emit_tri1(i)
                        emit_tri2(i)
                    emit_pv(i, 0)
                    emit_pv(i, 1)
                    emit_pv(i, 2)
                    if kk == 4 * qc + 3:
                        normalize(qc)
                for t in range(16):
                    proj_tile(t)

    nc.compile()
    return nc


def kernel(x, attn_mask, Wqkv, bqkv, Wproj, bproj):
    global last_results
    x = np.asarray(x, dtype=np.float32)
    attn_mask = np.asarray(attn_mask)
    Wqkv = np.asarray(Wqkv, dtype=np.float32)
    bqkv = np.asarray(bqkv, dtype=np.float32)
    Wproj = np.asarray(Wproj, dtype=np.float32)
    bproj = np.asarray(bproj, dtype=np.float32)
    bf = ml_dtypes.bfloat16

    if "nc" not in _cache:
        _cache["nc"] = _build()
    nc = _cache["nc"]

    in_maps = []
    for c in range(8):
        b, g = c // 4, c % 4
        cs = slice(192 * g, 192 * g + 192)
        wq = Wqkv[:, 0:768][:, cs] * 0.125
        bq = bqkv[0:768][cs] * 0.125
        wk = Wqkv[:, 768:1536][:, cs]
        bk = bqkv[768:1536][cs]
        wv = Wqkv[:, 1536:2304][:, cs]
        bv = bqkv[1536:2304][cs]
        wqk = np.concatenate([wq[:, 0:128], wk[:, 0:128],
                              wq[:, 128:192], wk[:, 128:192]], axis=1)
        bqk = np.zeros((128, 3), np.float32)
        bqk[:, 0] = bq[0:128]
        bqk[:, 1] = bk[0:128]
        bqk[0:64, 2] = bq[128:192]
        bqk[64:128, 2] = bk[128:192]
        maskcol = np.ascontiguousarray(
            (attn_mask[b].astype(np.float32).reshape(NT, 128).T - 1.0) * 1e9)
        smalls = np.concatenate([
            bqk, maskcol,
            np.broadcast_to(bv[None, :], (128, 192)),
        ], axis=1).astype(np.float32)
        xbT = np.ascontiguousarray(x[b].T).astype(bf)
        in_maps.append({
            **{f"xs{n}": np.ascontiguousarray(xbT[:, n * 512:(n + 1) * 512]) for n in range(4)},
            "wqk": np.ascontiguousarray(wqk).astype(bf),
            "wv": np.ascontiguousarray(wv).astype(bf),
            "wp": np.ascontiguousarray(Wproj[cs, :]).astype(bf),
            "smalls": np.ascontiguousarray(smalls),
        })

    trace = bool(int(os.environ.get("KERNEL_TRACE", "0")))
    res = run_bass_kernel_spmd(nc, in_maps, core_ids=list(range(8)), trace=trace)
    last_results = res

    parts = [np.asarray(res.results[c]["y"], dtype=np.float32) for c in range(8)]
    out = np.stack([
        parts[0] + parts[1] + parts[2] + parts[3],
        parts[4] + parts[5] + parts[6] + parts[7],
    ]) + bproj
    return out.astype(np.float32)


# revision 21
# speedup vs baseline: 1.1884x; 1.1884x over previous
"""Causal multi-head self-attention on 8 Trainium2 NeuronCores (Bass/Tile).

Problem (hardcoded shapes): x [2, 2048, 768] f32, 12 heads of dim 64.
    qkv = x @ Wqkv + bqkv ; per-head causal softmax(q k^T / 8) @ v ; out @ Wproj + bproj

Sharding: 8 cores = 2 batches x 4 head-groups (3 heads each). Each core computes
its heads' QKV, attention, and a partial output projection (its rows of Wproj).
Host sums the 4 partial projections per batch and adds bproj.

v2 design (all matmuls bf16, softmax-pipelined):
  - Inputs arrive bf16 (host-cast); x pre-transposed to xT [768, 2048].
  - qkT chunks {q0q1, k0k1, q2k2} each [128, 2048] via 6-step PSUM chains;
    PSUM->SBUF move on DVE carries the bias add (bf16 out).
  - v natural [2048, 3*65] (ones column for row sums), bias in the DVE move.
  - Attention, query-block 512, keys tiled by 128 (round r = (qc, kk)):
      scores^T on PE (h0/h1 at partition offsets 0/64 -> row-group paired,
      near-concurrent), ONE exp per score tile on ACT (bias = key mask),
      causal tri mask on GpSimd (diag rounds), PV accumulation into [65,512]
      PSUM per head.  Software pipelined so the PE never waits at queue head:
      per round emit scores(r), pv01(r-1), s1(r+1), pv2(r).
  - Background GEMM chains (remaining qk/v slices, projection tiles) are
    dispensed into the attention rounds' PE slack.
  - Projection y = A @ Wp accumulated in PSUM, moved to SBUF as bf16, DMA'd
    out per seq tile; host upcasts, sums the 4 partial projections, adds bproj.
PSUM: sA [128,1024] + sB [128,512] single-buffered, 3x oacc [65,512], 2 bg.
"""
import os
import numpy as np
import ml_dtypes

import concourse.bass as bass
import concourse.mybir as mybir
import concourse.tile as tile
from concourse import bacc
from concourse.bass_utils import run_bass_kernel_spmd
from concourse.masks import make_upper_triangular

f32 = mybir.dt.float32
f32r = mybir.dt.float32r
bf16 = mybir.dt.bfloat16

T = 2048          # sequence length
H = 768           # model dim
HD = 64           # head dim
NT = T // 128     # 16 key tiles
KH = H // 128     # 6 contraction chunks

_cache = {}
last_results = None


def _build():
    nc = bacc.Bacc("TRN2", target_bir_lowering=False, debug=False)

    xs_d = [nc.dram_tensor(f"xs{n}", [H, 512], bf16, kind="ExternalInput")
            for n in range(4)]
    wqk_d = nc.dram_tensor("wqk", [H, 384], bf16, kind="ExternalInput")
    wv_d = nc.dram_tensor("wv", [H, 192], bf16, kind="ExternalInput")
    wp_d = nc.dram_tensor("wp", [192, H], bf16, kind="ExternalInput")
    smalls_d = nc.dram_tensor("smalls", [128, 3 + NT + 192], f32, kind="ExternalInput")
    y_d = nc.dram_tensor("y", [T, H], bf16, kind="ExternalOutput")

    Exp = mybir.ActivationFunctionType.Exp
    mult = mybir.AluOpType.mult
    add = mybir.AluOpType.add

    with tile.TileContext(nc) as tc:
        with (
            tc.tile_pool(name="singles", bufs=1) as singles,
            tc.tile_pool(name="big", bufs=1) as big,
            tc.tile_pool(name="pt", bufs=2) as ptp,
            tc.tile_pool(name="small", bufs=2) as small,
            tc.tile_pool(name="yout", bufs=2) as ytp,
            tc.tile_pool(name="sA", bufs=1, space="PSUM") as sAp,
            tc.tile_pool(name="sB", bufs=1, space="PSUM") as sBp,
            tc.tile_pool(name="acc", bufs=3, space="PSUM") as accp,
            tc.tile_pool(name="bg", bufs=2, space="PSUM") as bgp,
        ):
            # ---- input DMAs: one queue, ordered by first need ----
            smalls = singles.tile([128, 3 + NT + 192], f32)
            nc.sync.dma_start(out=smalls[:], in_=smalls_d.ap())
            bqk = smalls[:, 0:3]
            maskcolD = smalls[:, 3:3 + NT]
            bvb = smalls[:, 3 + NT:3 + NT + 192]
            maskcol = singles.tile([128, NT], f32)
            nc.vector.tensor_copy(maskcol[:], maskcolD)
            wqk = singles.tile([128, KH, 384], bf16)
            nc.sync.dma_start(out=wqk[:], in_=wqk_d.ap().rearrange("(a p) m -> p a m", p=128))
            xT = big.tile([128, KH, T], bf16)
            wv = singles.tile([128, KH, 192], bf16)
            wp0 = singles.tile([128, H], bf16)
            wp1 = singles.tile([64, H], bf16)
            nc.sync.dma_start(out=xT[:, :, 0:512],
                              in_=xs_d[0].ap().rearrange("(a p) m -> p a m", p=128))
            nc.sync.dma_start(out=wv[:], in_=wv_d.ap().rearrange("(a p) m -> p a m", p=128))
            for n in range(1, 4):
                nc.sync.dma_start(out=xT[:, :, n * 512:(n + 1) * 512],
                                  in_=xs_d[n].ap().rearrange("(a p) m -> p a m", p=128))
            nc.sync.dma_start(out=wp0[:], in_=wp_d.ap()[0:128, :])
            nc.sync.dma_start(out=wp1[:], in_=wp_d.ap()[128:192, :])

            # ---- constants ----
            trif = singles.tile([128, 128], f32)
            make_upper_triangular(nc, trif[:], val=1.0, diag=True)
            tri01 = singles.tile([128, 128], bf16)
            nc.vector.tensor_copy(tri01[:], trif[:])
            ones128 = singles.tile([128, 1], bf16)
            nc.vector.memset(ones128[:], 1.0)
            onesr1 = singles.tile([1, 64], f32)
            nc.vector.memset(onesr1[:], 1.0)

            # ---- persistent activations ----
            qkA = big.tile([128, T], bf16)    # q0 (rows 0:64), q1 (64:128)
            qkB = big.tile([128, T], bf16)    # k0, k1
            qkC2 = big.tile([128, T], bf16)   # q2 (0:64), k2 (64:128)
            k2c = big.tile([64, T], bf16)     # k2 moved to partitions 0:64
            vsb = big.tile([128, NT, 3, HD + 1], bf16)
            nc.vector.tensor_copy(vsb[:, :, :, HD:HD + 1],
                                  ones128[:, 0:1].to_broadcast((128, NT, 3, 1)))
            atA = big.tile([128, T], bf16)    # A^T h0 (0:64), h1 (64:128)
            atB = big.tile([64, T], bf16)     # h2

            # ---- chain emitters ----
            def qk_chain(ci, n, dst):
                ns = slice(n * 512, (n + 1) * 512)
                ps = bgp.tile([128, 512], f32, tag="bg")
                for k in range(KH):
                    nc.tensor.matmul(ps[:], lhsT=wqk[:, k, ci * 128:(ci + 1) * 128],
                                     rhs=xT[:, k, ns], start=(k == 0), stop=(k == KH - 1))
                nc.vector.tensor_tensor(dst[:, ns], ps[:],
                                        bqk[:, ci:ci + 1].to_broadcast((128, 512)), add)

            def k2copy(n):
                ns = slice(n * 512, (n + 1) * 512)
                nc.vector.tensor_copy(k2c[:, ns], qkC2[64:128, ns])

            def v_chain(t):
                ps = bgp.tile([128, 512], f32, tag="bg")
                for k in range(KH):
                    nc.tensor.matmul(ps[:, 0:192], lhsT=xT[:, k, t * 128:(t + 1) * 128],
                                     rhs=wv[:, k, :], start=(k == 0), stop=(k == KH - 1))
                nc.vector.tensor_tensor(
                    vsb[:, t, :, 0:HD],
                    ps[:, 0:192].rearrange("p (h d) -> p h d", h=3),
                    bvb.rearrange("p (h d) -> p h d", h=3), add)

            Copy = mybir.ActivationFunctionType.Copy

            def proj_tile(t, act_half=False):
                ts = slice(t * 128, (t + 1) * 128)
                yt = ytp.tile([128, H], bf16, tag="yt")
                for hi, hs in enumerate((slice(0, 384), slice(384, 768))):
                    ps = bgp.tile([128, 512], f32, tag="bg")
                    nc.tensor.matmul(ps[:, 0:384], lhsT=atA[:, ts], rhs=wp0[:, hs],
                                     start=True, stop=False)
                    nc.tensor.matmul(ps[:, 0:384], lhsT=atB[:, ts], rhs=wp1[:, hs],
                                     start=False, stop=True)
                    if act_half and hi == 0:
                        nc.scalar.activation(out=yt[:, hs], in_=ps[:, 0:384], func=Copy)
                    else:
                        nc.vector.tensor_copy(yt[:, hs], ps[:, 0:384])
                nc.sync.dma_start(out=y_d.ap()[ts, :], in_=yt[:])

            # ---- background work queue: (cost_ns, emit_fn) ----
            # KERNEL_BG_MASK: bit0 = qk chains + k2copies in bg, bit1 = v chains in bg
            bgm = int(os.environ.get("KERNEL_BG_MASK", "3"))
            bgq = []
            front = []
            (bgq if bgm & 2 else front).append((700, lambda: None))
            bgq.pop() if bgm & 2 else front.pop()
            for t in range(4):
                (bgq if bgm & 2 else front).append((700, lambda t=t: v_chain(t)))
            for n in range(1, 4):
                (bgq if bgm & 1 else front).append((1500, lambda n=n: qk_chain(0, n, qkA)))
                (bgq if bgm & 1 else front).append((1500, lambda n=n: qk_chain(1, n, qkB)))
                (bgq if bgm & 1 else front).append((1500, lambda n=n: qk_chain(2, n, qkC2)))
                (bgq if bgm & 1 else front).append((100, lambda n=n: k2copy(n)))
                for t in range(4 * n, 4 * n + 4):
                    (bgq if bgm & 2 else front).append((700, lambda t=t: v_chain(t)))

            # ---- HAM warm-up: dummy matmuls while input DMAs stream ----
            wsrc = singles.tile([128, 512], bf16)
            nc.vector.memset(wsrc[:], 0.25)
            for _w in range(6):
                wps = bgp.tile([128, 512], f32, tag="bg", name="warm")
                nc.tensor.matmul(wps[:], lhsT=wsrc[:, 0:128], rhs=wsrc[:],
                                 start=True, stop=True)

            # ---- front: A0/B0 interleaved per contraction chunk ----
            psA0 = bgp.tile([128, 512], f32, tag="bg", name="psA0")
            psB0 = bgp.tile([128, 512], f32, tag="bg", name="psB0")
            for k in range(KH):
                nc.tensor.matmul(psA0[:], lhsT=wqk[:, k, 0:128], rhs=xT[:, k, 0:512],
                                 start=(k == 0), stop=(k == KH - 1))
                nc.tensor.matmul(psB0[:], lhsT=wqk[:, k, 128:256], rhs=xT[:, k, 0:512],
                                 start=(k == 0), stop=(k == KH - 1))
            nc.vector.tensor_tensor(qkA[:, 0:512], psA0[:],
                                    bqk[:, 0:1].to_broadcast((128, 512)), add)
            nc.vector.tensor_tensor(qkB[:, 0:512], psB0[:],
                                    bqk[:, 1:2].to_broadcast((128, 512)), add)
            for _c, _fn in front:
                _fn()

            # ---- attention rounds ----
            rounds = [(qc, kk) for qc in range(4) for kk in range(4 * qc + 4)]
            NR = len(rounds)
            sAt = [None] * NR
            sBt = [None] * NR
            pt2t = [None] * NR
            pt1t = [None] * NR
            oaccs = [None, None, None]

            def off_of(i):
                qc, kk = rounds[i]
                return max(0, kk * 128 - qc * 512)

            def emit_s1(i):
                qc, kk = rounds[i]
                base, qlo, off = qc * 512, kk * 128, off_of(i)
                sBt[i] = sBp.tile([128, 512], f32, tag="sB", name="sB")
                nc.tensor.matmul(sBt[i][:, off:512], lhsT=k2c[:, qlo:qlo + 128],
                                 rhs=qkC2[0:64, base + off:base + 512],
                                 start=True, stop=True)

            def emit_s2(i):
                qc, kk = rounds[i]
                base, qlo, off = qc * 512, kk * 128, off_of(i)
                sAt[i] = sAp.tile([128, 1024], f32, tag="sA", name="sA")
                nc.tensor.matmul(sAt[i][:, off:512], lhsT=qkB[0:64, qlo:qlo + 128],
                                 rhs=qkA[0:64, base + off:base + 512],
                                 start=True, stop=True)
                nc.tensor.matmul(sAt[i][:, 512 + off:1024], lhsT=qkB[64:128, qlo:qlo + 128],
                                 rhs=qkA[64:128, base + off:base + 512],
                                 start=True, stop=True)

            def emit_exps(i):
                qc, kk = rounds[i]
                off = off_of(i)
                pt2t[i] = ptp.tile([128, 1024], bf16, tag="pt2", name="pt2")
                nc.scalar.activation(out=pt2t[i][:, off:1024], in_=sAt[i][:, off:1024],
                                     func=Exp, bias=maskcol[:, kk:kk + 1], scale=1.0)
                pt1t[i] = ptp.tile([128, 512], bf16, tag="pt1", name="pt1")
                nc.scalar.activation(out=pt1t[i][:, off:512], in_=sBt[i][:, off:512],
                                     func=Exp, bias=maskcol[:, kk:kk + 1], scale=1.0)

            def is_diag(i):
                qc, kk = rounds[i]
                return kk * 128 >= qc * 512

            def emit_tri1(i):
                d = off_of(i)
                nc.vector.tensor_tensor(pt1t[i][:, d:d + 128], pt1t[i][:, d:d + 128],
                                        tri01[:], mult)

            def emit_tri2(i):
                d = off_of(i)
                for o2 in (0, 512):
                    nc.vector.tensor_tensor(pt2t[i][:, o2 + d:o2 + d + 128],
                                            pt2t[i][:, o2 + d:o2 + d + 128],
                                            tri01[:], mult)

            def emit_pv(i, h):
                qc, kk = rounds[i]
                off = off_of(i)
                pt = pt1t[i] if h == 2 else pt2t[i]
                o2 = 512 if h == 1 else 0
                nc.tensor.matmul(oaccs[h][0:HD + 1, off:512],
                                 lhsT=vsb[:, kk, h, :],
                                 rhs=pt[:, o2 + off:o2 + 512] if h != 2 else pt[:, off:512],
                                 start=(kk == 0), stop=(kk == 4 * qc + 3),
                                 skip_group_check=True)

            def new_oaccs():
                for h in range(3):
                    oaccs[h] = accp.tile([HD + 1, 512], f32, tag="acc", name=f"oacc{h}")

            at_of = [(atA, 0), (atA, 64), (atB, 0)]

            def norm_pre(qc):
                rbss = []
                for h in (0, 1, 2):
                    srow = small.tile([1, 512], f32, tag="srow", name=f"srow{h}")
                    nc.vector.tensor_copy(srow[:], oaccs[h][HD:HD + 1, :])
                    rrow = small.tile([1, 512], f32, tag="rrow", name=f"rrow{h}")
                    nc.vector.reciprocal_approx_fast(rrow[:], srow[:])
                    rbs = small.tile([64, 512], f32, tag=f"rbs{h}", name=f"rbs{h}")
                    nc.gpsimd.partition_broadcast(rbs[:], rrow[:])
                    rbss.append((h, rbs))
                return rbss

            def norm_mul(qc, rbss):
                base = qc * 512
                for h, rbs in rbss:
                    at_t, at_o = at_of[h]
                    nc.vector.tensor_tensor(at_t[at_o:at_o + HD, base:base + 512],
                                            oaccs[h][0:HD, :], rbs[:], mult)

            def normalize(qc):
                norm_mul(qc, norm_pre(qc))

            debt = [0.0]

            def drain_bg(budget):
                debt[0] += budget
                while bgq and debt[0] >= bgq[0][0]:
                    cost, fn = bgq.pop(0)
                    debt[0] -= cost
                    fn()

            if int(os.environ.get("KERNEL_PIPELINE", "1")):
                if int(os.environ.get("KERNEL_NOBG", "0")):
                    drain_bg(1e9)
                new_oaccs()
                for i in range(NR):
                    qc, kk = rounds[i]
                    first_of_qc = (kk == 0)
                    emit_s2(i)
                    if i == 0:
                        qk_chain(2, 0, qkC2)
                        k2copy(0)
                        emit_s1(0)
                    if first_of_qc and qc > 0:
                        emit_pv(i - 1, 0)
                        emit_pv(i - 1, 1)
                        emit_pv(i - 1, 2)
                        normalize(qc - 1)
                        for t in range(4 * (qc - 1), 4 * qc):
                            bgq.append((900, lambda t=t: proj_tile(t)))
                        new_oaccs()
                    emit_exps(i)
                    if is_diag(i):
                        emit_tri1(i)
                    drain_bg(2600 if i < 4 else (800 if i >= 24 else 600))
                    if i > 0 and not (first_of_qc and qc > 0):
                        emit_pv(i - 1, 0)
                        emit_pv(i - 1, 1)
                        emit_pv(i - 1, 2)
                    if i + 1 < NR:
                        emit_s1(i + 1)
                    if is_diag(i):
                        emit_tri2(i)
                emit_pv(NR - 1, 0)
                emit_pv(NR - 1, 1)
                emit_pv(NR - 1, 2)
                rbss3 = norm_pre(3)
                drain_bg(1e9)
                for _w in range(6):
                    wps = bgp.tile([128, 512], f32, tag="bg", name="warm2")
                    nc.tensor.matmul(wps[:], lhsT=wsrc[:, 0:128], rhs=wsrc[:],
                                     start=True, stop=True)
                norm_mul(3, rbss3)
                for t in range(12, 16):
                    proj_tile(t, act_half=True)
            else:
                # sequential debug mode: all chains first, plain rounds
                drain_bg(1e9)
                for i in range(NR):
                    qc, kk = rounds[i]
                    if kk == 0:
                        new_oaccs()
                    emit_s2(i)
                    emit_s1(i)
                    emit_exps(i)
                    if is_diag(i):
                        emit_tri1(i)
                        emit_tri2(i)
                    emit_pv(i, 0)
                    emit_pv(i, 1)
                    emit_pv(i, 2)
                    if kk == 4 * qc + 3:
                        normalize(qc)
                for t in range(16):
                    proj_tile(t)

    nc.compile()
    return nc


def kernel(x, attn_mask, Wqkv, bqkv, Wproj, bproj):
    global last_results
    x = np.asarray(x, dtype=np.float32)
    attn_mask = np.asarray(attn_mask)
    Wqkv = np.asarray(Wqkv, dtype=np.float32)
    bqkv = np.asarray(bqkv, dtype=np.float32)
    Wproj = np.asarray(Wproj, dtype=np.float32)
    bproj = np.asarray(bproj, dtype=np.float32)
    bf = ml_dtypes.bfloat16

    if "nc" not in _cache:
        _cache["nc"] = _build()
    nc = _cache["nc"]

    in_maps = []
    for c in range(8):
        b, g = c // 4, c % 4
        cs = slice(192 * g, 192 * g + 192)
        wq = Wqkv[:, 0:768][:, cs] * 0.125
        bq = bqkv[0:768][cs] * 0.125
        wk = Wqkv[:, 768:1536][:, cs]
        bk = bqkv[768:1536][cs]
        wv = Wqkv[:, 1536:2304][:, cs]
        bv = bqkv[1536:2304][cs]
        wqk = np.concatenate([wq[:, 0:128], wk[:, 0:128],
                              wq[:, 128:192], wk[:, 128:192]], axis=1)
        bqk = np.zeros((128, 3), np.float32)
        bqk[:, 0] = bq[0:128]
        bqk[:, 1] = bk[0:128]
        bqk[0:64, 2] = bq[128:192]
        bqk[64:128, 2] = bk[128:192]
        maskcol = np.ascontiguousarray(
            (attn_mask[b].astype(np.float32).reshape(NT, 128).T - 1.0) * 1e9)
        smalls = np.concatenate([
            bqk, maskcol,
            np.broadcast_to(bv[None, :], (128, 192)),
        ], axis=1).astype(np.float32)
        xbT = np.ascontiguousarray(x[b].T).astype(bf)
        in_maps.append({
            **{f"xs{n}": np.ascontiguousarray(xbT[:, n * 512:(n + 1) * 512]) for n in range(4)},
            "wqk": np.ascontiguousarray(wqk).astype(bf),
            "wv": np.ascontiguousarray(wv).astype(bf),
            "wp": np.ascontiguousarray(Wproj[cs, :]).astype(bf),
            "smalls": np.ascontiguousarray(smalls),
        })

    trace = bool(int(os.environ.get("KERNEL_TRACE", "0")))
    res = run_bass_kernel_spmd(nc, in_maps, core_ids=list(range(8)), trace=trace)
    last_results = res

    parts = [np.asarray(res.results[c]["y"], dtype=np.float32) for c in range(8)]
    out = np.stack([
        parts[0] + parts[1] + parts[2] + parts[3],
        parts[4] + parts[5] + parts[6] + parts[7],
    ]) + bproj
    return out.astype(np.float32)


# revision 22
# speedup vs baseline: 1.2003x; 1.0100x over previous
"""Causal multi-head self-attention on 8 Trainium2 NeuronCores (Bass/Tile).

Problem (hardcoded shapes): x [2, 2048, 768] f32, 12 heads of dim 64.
    qkv = x @ Wqkv + bqkv ; per-head causal softmax(q k^T / 8) @ v ; out @ Wproj + bproj

Sharding: 8 cores = 2 batches x 4 head-groups (3 heads each). Each core computes
its heads' QKV, attention, and a partial output projection (its rows of Wproj).
Host sums the 4 partial projections per batch and adds bproj.

v2 design (all matmuls bf16, softmax-pipelined):
  - Inputs arrive bf16 (host-cast); x pre-transposed to xT [768, 2048].
  - qkT chunks {q0q1, k0k1, q2k2} each [128, 2048] via 6-step PSUM chains;
    PSUM->SBUF move on DVE carries the bias add (bf16 out).
  - v natural [2048, 3*65] (ones column for row sums), bias in the DVE move.
  - Attention, query-block 512, keys tiled by 128 (round r = (qc, kk)):
      scores^T on PE (h0/h1 at partition offsets 0/64 -> row-group paired,
      near-concurrent), ONE exp per score tile on ACT (bias = key mask),
      causal tri mask on GpSimd (diag rounds), PV accumulation into [65,512]
      PSUM per head.  Software pipelined so the PE never waits at queue head:
      per round emit scores(r), pv01(r-1), s1(r+1), pv2(r).
  - Background GEMM chains (remaining qk/v slices, projection tiles) are
    dispensed into the attention rounds' PE slack.
  - Projection y = A @ Wp accumulated in PSUM, moved to SBUF as bf16, DMA'd
    out per seq tile; host upcasts, sums the 4 partial projections, adds bproj.
PSUM: sA [128,1024] + sB [128,512] single-buffered, 3x oacc [65,512], 2 bg.
"""
import os
import numpy as np
import ml_dtypes

import concourse.bass as bass
import concourse.mybir as mybir
import concourse.tile as tile
from concourse import bacc
from concourse.bass_utils import run_bass_kernel_spmd
from concourse.masks import make_upper_triangular

f32 = mybir.dt.float32
f32r = mybir.dt.float32r
bf16 = mybir.dt.bfloat16

T = 2048          # sequence length
H = 768           # model dim
HD = 64           # head dim
NT = T // 128     # 16 key tiles
KH = H // 128     # 6 contraction chunks

_cache = {}
last_results = None


def _build():
    nc = bacc.Bacc("TRN2", target_bir_lowering=False, debug=False)

    xs_d = [nc.dram_tensor(f"xs{n}", [H, 512], bf16, kind="ExternalInput")
            for n in range(4)]
    wqk_d = nc.dram_tensor("wqk", [H, 384], bf16, kind="ExternalInput")
    wv_d = nc.dram_tensor("wv", [H, 192], bf16, kind="ExternalInput")
    wp_d = nc.dram_tensor("wp", [192, H], bf16, kind="ExternalInput")
    smalls_d = nc.dram_tensor("smalls", [128, 3 + NT + 192], f32, kind="ExternalInput")
    y_d = nc.dram_tensor("y", [T, H], bf16, kind="ExternalOutput")

    Exp = mybir.ActivationFunctionType.Exp
    mult = mybir.AluOpType.mult
    add = mybir.AluOpType.add

    with tile.TileContext(nc) as tc:
        with (
            tc.tile_pool(name="singles", bufs=1) as singles,
            tc.tile_pool(name="big", bufs=1) as big,
            tc.tile_pool(name="pt", bufs=2) as ptp,
            tc.tile_pool(name="small", bufs=2) as small,
            tc.tile_pool(name="yout", bufs=2) as ytp,
            tc.tile_pool(name="sA", bufs=1, space="PSUM") as sAp,
            tc.tile_pool(name="sB", bufs=1, space="PSUM") as sBp,
            tc.tile_pool(name="acc", bufs=3, space="PSUM") as accp,
            tc.tile_pool(name="bg", bufs=2, space="PSUM") as bgp,
        ):
            # ---- input DMAs: one queue, ordered by first need ----
            smalls = singles.tile([128, 3 + NT + 192], f32)
            nc.sync.dma_start(out=smalls[:], in_=smalls_d.ap())
            bqk = smalls[:, 0:3]
            maskcolD = smalls[:, 3:3 + NT]
            bvb = smalls[:, 3 + NT:3 + NT + 192]
            maskcol = singles.tile([128, NT], f32)
            nc.vector.tensor_copy(maskcol[:], maskcolD)
            wqk = singles.tile([128, KH, 384], bf16)
            nc.sync.dma_start(out=wqk[:], in_=wqk_d.ap().rearrange("(a p) m -> p a m", p=128))
            xT = big.tile([128, KH, T], bf16)
            wv = singles.tile([128, KH, 192], bf16)
            wp0 = singles.tile([128, H], bf16)
            wp1 = singles.tile([64, H], bf16)
            nc.sync.dma_start(out=xT[:, :, 0:512],
                              in_=xs_d[0].ap().rearrange("(a p) m -> p a m", p=128))
            nc.sync.dma_start(out=wv[:], in_=wv_d.ap().rearrange("(a p) m -> p a m", p=128))
            for n in range(1, 4):
                nc.sync.dma_start(out=xT[:, :, n * 512:(n + 1) * 512],
                                  in_=xs_d[n].ap().rearrange("(a p) m -> p a m", p=128))
            nc.sync.dma_start(out=wp0[:], in_=wp_d.ap()[0:128, :])
            nc.sync.dma_start(out=wp1[:], in_=wp_d.ap()[128:192, :])

            # ---- constants ----
            trif = singles.tile([128, 128], f32)
            make_upper_triangular(nc, trif[:], val=1.0, diag=True)
            tri01 = singles.tile([128, 128], bf16)
            nc.vector.tensor_copy(tri01[:], trif[:])
            ones128 = singles.tile([128, 1], bf16)
            nc.vector.memset(ones128[:], 1.0)
            onesr1 = singles.tile([1, 64], f32)
            nc.vector.memset(onesr1[:], 1.0)

            # ---- persistent activations ----
            qkA = big.tile([128, T], bf16)    # q0 (rows 0:64), q1 (64:128)
            qkB = big.tile([128, T], bf16)    # k0, k1
            qkC2 = big.tile([128, T], bf16)   # q2 (0:64), k2 (64:128)
            k2c = big.tile([64, T], bf16)     # k2 moved to partitions 0:64
            vsb = big.tile([128, NT, 3, HD + 1], bf16)
            nc.vector.tensor_copy(vsb[:, :, :, HD:HD + 1],
                                  ones128[:, 0:1].to_broadcast((128, NT, 3, 1)))
            atA = big.tile([128, T], bf16)    # A^T h0 (0:64), h1 (64:128)
            atB = big.tile([64, T], bf16)     # h2

            # ---- chain emitters ----
            def qk_chain(ci, n, dst):
                ns = slice(n * 512, (n + 1) * 512)
                ps = bgp.tile([128, 512], f32, tag="bg")
                for k in range(KH):
                    nc.tensor.matmul(ps[:], lhsT=wqk[:, k, ci * 128:(ci + 1) * 128],
                                     rhs=xT[:, k, ns], start=(k == 0), stop=(k == KH - 1))
                nc.vector.tensor_tensor(dst[:, ns], ps[:],
                                        bqk[:, ci:ci + 1].to_broadcast((128, 512)), add)

            def k2copy(n):
                ns = slice(n * 512, (n + 1) * 512)
                nc.vector.tensor_copy(k2c[:, ns], qkC2[64:128, ns])

            def v_chain(t):
                ps = bgp.tile([128, 512], f32, tag="bg")
                for k in range(KH):
                    nc.tensor.matmul(ps[:, 0:192], lhsT=xT[:, k, t * 128:(t + 1) * 128],
                                     rhs=wv[:, k, :], start=(k == 0), stop=(k == KH - 1))
                nc.vector.tensor_tensor(
                    vsb[:, t, :, 0:HD],
                    ps[:, 0:192].rearrange("p (h d) -> p h d", h=3),
                    bvb.rearrange("p (h d) -> p h d", h=3), add)

            Copy = mybir.ActivationFunctionType.Copy

            def proj_tile(t, act_half=False):
                ts = slice(t * 128, (t + 1) * 128)
                yt = ytp.tile([128, H], bf16, tag="yt")
                for hi, hs in enumerate((slice(0, 384), slice(384, 768))):
                    ps = bgp.tile([128, 512], f32, tag="bg")
                    nc.tensor.matmul(ps[:, 0:384], lhsT=atA[:, ts], rhs=wp0[:, hs],
                                     start=True, stop=False)
                    nc.tensor.matmul(ps[:, 0:384], lhsT=atB[:, ts], rhs=wp1[:, hs],
                                     start=False, stop=True)
                    if act_half and hi == 0:
                        nc.scalar.activation(out=yt[:, hs], in_=ps[:, 0:384], func=Copy)
                    else:
                        nc.vector.tensor_copy(yt[:, hs], ps[:, 0:384])
                nc.sync.dma_start(out=y_d.ap()[ts, :], in_=yt[:])

            # ---- background work queue: (cost_ns, emit_fn) ----
            # KERNEL_BG_MASK: bit0 = qk chains + k2copies in bg, bit1 = v chains in bg
            bgm = int(os.environ.get("KERNEL_BG_MASK", "3"))
            bgq = []
            front = []
            (bgq if bgm & 2 else front).append((700, lambda: None))
            bgq.pop() if bgm & 2 else front.pop()
            for t in range(4):
                (bgq if bgm & 2 else front).append((700, lambda t=t: v_chain(t)))
            for n in range(1, 4):
                (bgq if bgm & 1 else front).append((1500, lambda n=n: qk_chain(0, n, qkA)))
                (bgq if bgm & 1 else front).append((1500, lambda n=n: qk_chain(1, n, qkB)))
                (bgq if bgm & 1 else front).append((1500, lambda n=n: qk_chain(2, n, qkC2)))
                (bgq if bgm & 1 else front).append((100, lambda n=n: k2copy(n)))
                for t in range(4 * n, 4 * n + 4):
                    (bgq if bgm & 2 else front).append((700, lambda t=t: v_chain(t)))

            # ---- HAM warm-up: dummy matmuls while input DMAs stream ----
            wsrc = singles.tile([128, 512], bf16)
            nc.vector.memset(wsrc[:], 0.25)
            for _w in range(12):
                wps = bgp.tile([128, 512], f32, tag="bg", name="warm")
                nc.tensor.matmul(wps[:], lhsT=wsrc[:, 0:128], rhs=wsrc[:],
                                 start=True, stop=True)

            # ---- front: A0/B0 interleaved per contraction chunk ----
            psA0 = bgp.tile([128, 512], f32, tag="bg", name="psA0")
            psB0 = bgp.tile([128, 512], f32, tag="bg", name="psB0")
            for k in range(KH):
                nc.tensor.matmul(psA0[:], lhsT=wqk[:, k, 0:128], rhs=xT[:, k, 0:512],
                                 start=(k == 0), stop=(k == KH - 1))
                nc.tensor.matmul(psB0[:], lhsT=wqk[:, k, 128:256], rhs=xT[:, k, 0:512],
                                 start=(k == 0), stop=(k == KH - 1))
            nc.vector.tensor_tensor(qkA[:, 0:512], psA0[:],
                                    bqk[:, 0:1].to_broadcast((128, 512)), add)
            nc.vector.tensor_tensor(qkB[:, 0:512], psB0[:],
                                    bqk[:, 1:2].to_broadcast((128, 512)), add)
            for _c, _fn in front:
                _fn()

            # ---- attention rounds ----
            rounds = [(qc, kk) for qc in range(4) for kk in range(4 * qc + 4)]
            NR = len(rounds)
            sAt = [None] * NR
            sBt = [None] * NR
            pt2t = [None] * NR
            pt1t = [None] * NR
            oaccs = [None, None, None]

            def off_of(i):
                qc, kk = rounds[i]
                return max(0, kk * 128 - qc * 512)

            def emit_s1(i):
                qc, kk = rounds[i]
                base, qlo, off = qc * 512, kk * 128, off_of(i)
                sBt[i] = sBp.tile([128, 512], f32, tag="sB", name="sB")
                nc.tensor.matmul(sBt[i][:, off:512], lhsT=k2c[:, qlo:qlo + 128],
                                 rhs=qkC2[0:64, base + off:base + 512],
                                 start=True, stop=True)

            def emit_s2(i):
                qc, kk = rounds[i]
                base, qlo, off = qc * 512, kk * 128, off_of(i)
                sAt[i] = sAp.tile([128, 1024], f32, tag="sA", name="sA")
                nc.tensor.matmul(sAt[i][:, off:512], lhsT=qkB[0:64, qlo:qlo + 128],
                                 rhs=qkA[0:64, base + off:base + 512],
                                 start=True, stop=True)
                nc.tensor.matmul(sAt[i][:, 512 + off:1024], lhsT=qkB[64:128, qlo:qlo + 128],
                                 rhs=qkA[64:128, base + off:base + 512],
                                 start=True, stop=True)

            def emit_exps(i):
                qc, kk = rounds[i]
                off = off_of(i)
                pt2t[i] = ptp.tile([128, 1024], bf16, tag="pt2", name="pt2")
                nc.scalar.activation(out=pt2t[i][:, off:1024], in_=sAt[i][:, off:1024],
                                     func=Exp, bias=maskcol[:, kk:kk + 1], scale=1.0)
                pt1t[i] = ptp.tile([128, 512], bf16, tag="pt1", name="pt1")
                nc.scalar.activation(out=pt1t[i][:, off:512], in_=sBt[i][:, off:512],
                                     func=Exp, bias=maskcol[:, kk:kk + 1], scale=1.0)

            def is_diag(i):
                qc, kk = rounds[i]
                return kk * 128 >= qc * 512

            def emit_tri1(i):
                d = off_of(i)
                nc.vector.tensor_tensor(pt1t[i][:, d:d + 128], pt1t[i][:, d:d + 128],
                                        tri01[:], mult)

            def emit_tri2(i):
                d = off_of(i)
                for o2 in (0, 512):
                    nc.vector.tensor_tensor(pt2t[i][:, o2 + d:o2 + d + 128],
                                            pt2t[i][:, o2 + d:o2 + d + 128],
                                            tri01[:], mult)

            def emit_pv(i, h):
                qc, kk = rounds[i]
                off = off_of(i)
                pt = pt1t[i] if h == 2 else pt2t[i]
                o2 = 512 if h == 1 else 0
                nc.tensor.matmul(oaccs[h][0:HD + 1, off:512],
                                 lhsT=vsb[:, kk, h, :],
                                 rhs=pt[:, o2 + off:o2 + 512] if h != 2 else pt[:, off:512],
                                 start=(kk == 0), stop=(kk == 4 * qc + 3),
                                 skip_group_check=True)

            def new_oaccs():
                for h in range(3):
                    oaccs[h] = accp.tile([HD + 1, 512], f32, tag="acc", name=f"oacc{h}")

            at_of = [(atA, 0), (atA, 64), (atB, 0)]

            def norm_pre(qc):
                rbss = []
                for h in (0, 1, 2):
                    srow = small.tile([1, 512], f32, tag="srow", name=f"srow{h}")
                    nc.vector.tensor_copy(srow[:], oaccs[h][HD:HD + 1, :])
                    rrow = small.tile([1, 512], f32, tag="rrow", name=f"rrow{h}")
                    nc.vector.reciprocal_approx_fast(rrow[:], srow[:])
                    rbs = small.tile([64, 512], f32, tag=f"rbs{h}", name=f"rbs{h}")
                    nc.gpsimd.partition_broadcast(rbs[:], rrow[:])
                    rbss.append((h, rbs))
                return rbss

            def norm_mul(qc, rbss):
                base = qc * 512
                for h, rbs in rbss:
                    at_t, at_o = at_of[h]
                    nc.vector.tensor_tensor(at_t[at_o:at_o + HD, base:base + 512],
                                            oaccs[h][0:HD, :], rbs[:], mult)

            def normalize(qc):
                norm_mul(qc, norm_pre(qc))

            debt = [0.0]

            def drain_bg(budget):
                debt[0] += budget
                while bgq and debt[0] >= bgq[0][0]:
                    cost, fn = bgq.pop(0)
                    debt[0] -= cost
                    fn()

            if int(os.environ.get("KERNEL_PIPELINE", "1")):
                if int(os.environ.get("KERNEL_NOBG", "0")):
                    drain_bg(1e9)
                new_oaccs()
                for i in range(NR):
                    qc, kk = rounds[i]
                    first_of_qc = (kk == 0)
                    emit_s2(i)
                    if i == 0:
                        qk_chain(2, 0, qkC2)
                        k2copy(0)
                    emit_s1(i)
                    if first_of_qc and qc > 0:
                        emit_pv(i - 1, 0)
                        emit_pv(i - 1, 1)
                        emit_pv(i - 1, 2)
                        normalize(qc - 1)
                        for t in range(4 * (qc - 1), 4 * qc):
                            bgq.append((900, lambda t=t: proj_tile(t)))
                        new_oaccs()
                    emit_exps(i)
                    if is_diag(i):
                        emit_tri1(i)
                    drain_bg(2600 if i < 4 else (800 if i >= 24 else 600))
                    if i > 0 and not (first_of_qc and qc > 0):
                        emit_pv(i - 1, 0)
                        emit_pv(i - 1, 1)
                        emit_pv(i - 1, 2)
                    if is_diag(i):
                        emit_tri2(i)
                emit_pv(NR - 1, 0)
                emit_pv(NR - 1, 1)
                emit_pv(NR - 1, 2)
                rbss3 = norm_pre(3)
                drain_bg(1e9)
                for _w in range(6):
                    wps = bgp.tile([128, 512], f32, tag="bg", name="warm2")
                    nc.tensor.matmul(wps[:], lhsT=wsrc[:, 0:128], rhs=wsrc[:],
                                     start=True, stop=True)
                norm_mul(3, rbss3)
                for t in range(12, 16):
                    proj_tile(t, act_half=True)
            else:
                # sequential debug mode: all chains first, plain rounds
                drain_bg(1e9)
                for i in range(NR):
                    qc, kk = rounds[i]
                    if kk == 0:
                        new_oaccs()
                    emit_s2(i)
                    emit_s1(i)
                    emit_exps(i)
                    if is_diag(i):
                        emit_tri1(i)
                        emit_tri2(i)
                    emit_pv(i, 0)
                    emit_pv(i, 1)
                    emit_pv(i, 2)
                    if kk == 4 * qc + 3:
                        normalize(qc)
                for t in range(16):
                    proj_tile(t)

    nc.compile()
    return nc


def kernel(x, attn_mask, Wqkv, bqkv, Wproj, bproj):
    global last_results
    x = np.asarray(x, dtype=np.float32)
    attn_mask = np.asarray(attn_mask)
    Wqkv = np.asarray(Wqkv, dtype=np.float32)
    bqkv = np.asarray(bqkv, dtype=np.float32)
    Wproj = np.asarray(Wproj, dtype=np.float32)
    bproj = np.asarray(bproj, dtype=np.float32)
    bf = ml_dtypes.bfloat16

    if "nc" not in _cache:
        _cache["nc"] = _build()
    nc = _cache["nc"]

    in_maps = []
    for c in range(8):
        b, g = c // 4, c % 4
        cs = slice(192 * g, 192 * g + 192)
        wq = Wqkv[:, 0:768][:, cs] * 0.125
        bq = bqkv[0:768][cs] * 0.125
        wk = Wqkv[:, 768:1536][:, cs]
        bk = bqkv[768:1536][cs]
        wv = Wqkv[:, 1536:2304][:, cs]
        bv = bqkv[1536:2304][cs]
        wqk = np.concatenate([wq[:, 0:128], wk[:, 0:128],
                              wq[:, 128:192], wk[:, 128:192]], axis=1)
        bqk = np.zeros((128, 3), np.float32)
        bqk[:, 0] = bq[0:128]
        bqk[:, 1] = bk[0:128]
        bqk[0:64, 2] = bq[128:192]
        bqk[64:128, 2] = bk[128:192]
        maskcol = np.ascontiguousarray(
            (attn_mask[b].astype(np.float32).reshape(NT, 128).T - 1.0) * 1e9)
        smalls = np.concatenate([
            bqk, maskcol,
            np.broadcast_to(bv[None, :], (128, 192)),
        ], axis=1).astype(np.float32)
        xbT = np.ascontiguousarray(x[b].T).astype(bf)
        in_maps.append({
            **{f"xs{n}": np.ascontiguousarray(xbT[:, n * 512:(n + 1) * 512]) for n in range(4)},
            "wqk": np.ascontiguousarray(wqk).astype(bf),
            "wv": np.ascontiguousarray(wv).astype(bf),
            "wp": np.ascontiguousarray(Wproj[cs, :]).astype(bf),
            "smalls": np.ascontiguousarray(smalls),
        })

    trace = bool(int(os.environ.get("KERNEL_TRACE", "0")))
    res = run_bass_kernel_spmd(nc, in_maps, core_ids=list(range(8)), trace=trace)
    last_results = res

    parts = [np.asarray(res.results[c]["y"], dtype=np.float32) for c in range(8)]
    out = np.stack([
        parts[0] + parts[1] + parts[2] + parts[3],
        parts[4] + parts[5] + parts[6] + parts[7],
    ]) + bproj
    return out.astype(np.float32)
